# revision 26
# baseline (speedup 1.0000x reference)
"""Trainium2 Bass kernel for nn_KuramotoHyperUniversal.

Data-parallel over batch across 8 NeuronCores (64 rows/core); weights
replicated. The (B,D,D) pairwise term is computed via the identity
  sum_j sin(y_j - y_i) A[i,j] = cos(y_i)*(A@sin(y))_i - sin(y_i)*(A@cos(y))_i
so it becomes two [64,512]x[512,512] matmuls instead of a 64MB tensor.
The constant t-column of the MLP input is folded into the layer-0 bias.

Memory-bound on weight streaming; the key optimizations vs a naive port:
- Weights and A are host-cast to bf16 (halves HBM bytes; matmuls run
  bf16 x bf16 with fp32 PSUM accumulation; rel err ~7.4e-3 vs 2e-2 gate).
- One full-width DMA per 128-row weight K-tile (394KB), alternated
  between the two HWDGE rings (SP via nc.sync, ACT via nc.scalar) —
  each dma_start costs ~0.6us of ring occupancy, so DMA count and
  single-ring serialization dominate before bandwidth does.
- Biases of layers 1-3 are appended as an extra row of W on the host,
  so the K-tail (rows 1536:1538 + bias) arrives in ONE small DMA that
  is issued BEFORE the layer's big tiles (ring FIFO: the group-closing
  tail matmul must not wait behind prefetched next-layer DMAs), and the
  bias-add folds into the tail matmul (the rhs tail tile carries a ones
  row produced by a ones-column appended to h before transposing).
- w_bufs=12 weight prefetch depth measured optimal (deeper prefetch
  delays current-layer tiles in the ring FIFO and is strictly worse).

Measured ~49-52us per invocation on-device (loop-slope method), vs
139.6us for the fp32r two-DMAs-per-tile single-ring baseline.
"""

import numpy as np
import ml_dtypes
from contextlib import ExitStack

import concourse.bass as bass
import concourse.mybir as mybir
import concourse.tile as tile
from concourse.vector_clock import ScopedClock, VectorClock
from concourse.bass_utils import run_bass_kernel_spmd
from concourse.masks import make_identity

DIM = 512
BATCH = 512
NCORES = 8
BS = BATCH // NCORES  # 64
H = 2 + 3 * DIM  # 1538
IN_SZ = 1 + 3 * DIM  # 1537
F32 = mybir.dt.float32
BF16 = mybir.dt.bfloat16
PI_HALF = float(np.pi / 2.0)


def _split_drain_and_barrier(self, tick_clock, wait_clock):
    # Walrus in this container rejects >2 sync waits on one CTRL (drain)
    # instruction; emit one single-wait NOP per outstanding proc instead.
    gc = tick_clock.global_clock
    ticks = list(gc)
    for p, t in enumerate(ticks):
        if t > 0:
            v = [0] * len(ticks)
            v[p] = t
            nop = self.nc.sync.nop(nofuse=True, hint=f"drain_wait_{p}")
            wait_clock.add_sem_waits(nop.ins, ScopedClock({None: VectorClock(v)}))
    self.nc.sync.drain()
    self.nc.all_engine_barrier()
    popped = self.nc._tile_sem_poison_stack.pop()
    assert popped is self._sem_poison
    self.nc.clear_and_free_semaphores(list(self.sems.allocated().values()))
    self.nc.all_engine_barrier()


tile.TileContext._drain_and_barrier = _split_drain_and_barrier


_MAX_WAITS = 1


def _split_waits(nc, limit=_MAX_WAITS):
    """Walrus rejects instructions carrying more than `limit` sync waits;
    move the excess onto same-engine NOPs inserted just before."""
    import bass_rust

    n = 0
    for f in nc.m.functions:
        for bb in f.blocks:
            out = []
            for inst in bb.instructions:
                si = inst.sync_info
                if si is not None and si.on_wait and len(si.on_wait) > limit:
                    waits = list(si.on_wait)
                    extra, keep = waits[:-limit], waits[-limit:]
                    for i in range(0, len(extra), limit):
                        nop = mybir.InstNoOp(name=f"I-wsplit-{n}", engine=inst.engine)
                        n += 1
                        nop.sync_info = bass_rust.SyncInfo(
                            on_wait=extra[i : i + limit], on_update=[]
                        )
                        out.append(nop)
                    inst.sync_info = bass_rust.SyncInfo(
                        on_wait=keep, on_update=list(si.on_update)
                    )
                out.append(inst)
            bb.instructions = out


def _build(w_bufs=12, reps=1, loop_reps=None):
    nc = bass.Bass()
    AF = mybir.ActivationFunctionType

    t_p = nc.declare_dram_parameter("t", [1], F32, isOutput=False)
    y_p = nc.declare_dram_parameter("y", [BS, DIM + 1], F32, isOutput=False)
    fr_p = nc.declare_dram_parameter("freqs", [BS, DIM], F32, isOutput=False)
    A_p = nc.declare_dram_parameter("A", [DIM, DIM], BF16, isOutput=False)
    # W1..W3 carry their bias appended as a final row (host-side vstack),
    # so the K-tail rows 1536:1539 (w1536, w1537, bias) arrive in one DMA
    # and the bias-add folds into the tail matmul (rhs tile has a ones row).
    W_p = [
        nc.declare_dram_parameter("W0", [IN_SZ, H], BF16, isOutput=False),
        nc.declare_dram_parameter("W1", [H + 1, H], BF16, isOutput=False),
        nc.declare_dram_parameter("W2", [H + 1, H], BF16, isOutput=False),
        nc.declare_dram_parameter("W3", [H + 1, DIM], BF16, isOutput=False),
    ]
    b0_p = nc.declare_dram_parameter("b0", [H], BF16, isOutput=False)
    out_p = nc.declare_dram_parameter("out", [BS, DIM + 1], F32, isOutput=True)

    with ExitStack() as ctx:
        tc = ctx.enter_context(tile.TileContext(nc))
        const = ctx.enter_context(tc.tile_pool(name="const", bufs=1))
        io = ctx.enter_context(tc.tile_pool(name="io", bufs=1))
        xtp = ctx.enter_context(tc.tile_pool(name="xtp", bufs=1))
        atp = ctx.enter_context(tc.tile_pool(name="atp", bufs=1))
        htp = ctx.enter_context(tc.tile_pool(name="htp", bufs=2))
        wp = ctx.enter_context(tc.tile_pool(name="wp", bufs=w_bufs))
        ain = ctx.enter_context(tc.tile_pool(name="ain", bufs=2))
        ps = ctx.enter_context(tc.tile_pool(name="ps", bufs=1, space="PSUM"))
        pst = ctx.enter_context(tc.tile_pool(name="pst", bufs=2, space="PSUM"))

        id64 = const.tile([64, 64], F32, tag="id64")
        make_identity(nc, id64[:])
        id64b = const.tile([64, 64], BF16, tag="id64b")
        nc.vector.tensor_copy(id64b[:], id64[:])
        id128f = const.tile([128, 128], F32, tag="id128f")
        make_identity(nc, id128f[:])
        id128b = const.tile([128, 128], BF16, tag="id128b")
        nc.vector.tensor_copy(id128b[:], id128f[:])
        ones = const.tile([1, 64], BF16, tag="ones")
        nc.vector.memset(ones[:], 1.0)
        pih = const.tile([BS, 1], F32, tag="pih")
        nc.vector.memset(pih[:], PI_HALF)

        # Alternate weight DMAs between the two HWDGE rings (SP via nc.sync,
        # ACT via nc.scalar) — each dma_start occupies its ring for ~0.6us
        # fixed cost, so a single ring serializes on DMA count.
        _ring = [0]

        def _dma(out, in_):
            eng = nc.sync if _ring[0] % 2 == 0 else nc.scalar
            _ring[0] += 1
            eng.dma_start(out=out, in_=in_)

        def _emit(rep):
            # ---- inputs ----
            yd = io.tile([BS, DIM], F32, tag="yd")
            nc.scalar.dma_start(out=yd[:], in_=y_p[:, 0:DIM])
            fr = io.tile([BS, DIM], F32, tag="fr")
            nc.scalar.dma_start(out=fr[:], in_=fr_p[:])
            t_sb = const.tile([1, 1], F32, tag="t")
            nc.scalar.dma_start(out=t_sb[:], in_=t_p[None, :])

            # C = cos(yd) = sin(yd + pi/2), S = sin(yd)   [64, 512]
            C = io.tile([BS, DIM], F32, tag="C")
            nc.scalar.activation(C[:], yd[:], AF.Sin, bias=pih[:])
            S = io.tile([BS, DIM], F32, tag="S")
            nc.scalar.activation(S[:], yd[:], AF.Sin)

            # ---- transposed copies (feature-on-partition, [128, 64] bf16) ----
            def transpose4(src, pref):
                tiles = []
                for j in range(4):
                    p = pst.tile([128, 64], F32, tag="pstT")
                    nc.tensor.transpose(p[:], src[:, j * 128 : (j + 1) * 128], id64[:])
                    tt = xtp.tile([128, 64], BF16, tag=f"{pref}{j}")
                    nc.vector.tensor_copy(tt[:], p[:])
                    tiles.append(tt)
                return tiles

            xC = transpose4(C, "xC")
            xS = transpose4(S, "xS")
            xF = transpose4(fr, "xF")

            # ---- biases: b0 row for the L0 ones-matmul; b1..b3 are folded
            # into the K-tail weight tiles inside mlp_layer ----
            b0row = const.tile([1, H], BF16, tag="b0", name="b0row")
            nc.scalar.dma_start(out=b0row[:], in_=b0_p[None, :])
            # b0' = b0 + (t-1) * W0[1024, :]
            w0row = const.tile([1, H], BF16, tag="w0row")
            nc.scalar.dma_start(out=w0row[:], in_=W_p[0][1024:1025, :])
            tm1 = const.tile([1, 1], F32, tag="tm1")
            nc.vector.tensor_scalar_add(tm1[:], t_sb[:], -1.0)
            b0p = const.tile([1, H], BF16, tag="b0p")
            nc.vector.tensor_scalar_mul(b0p[:], w0row[:], tm1[:])
            nc.vector.tensor_add(b0p[:], b0p[:], b0row[:])

            # ---- MLP ----
            def mlp_layer(l, in_tiles, in_tail, out_dim, bias, act_fn, out_dt):
                """in_tiles: list of (xT_tile[128,64] bf16, W row offset);
                in_tail: ([2,64] tile, row offset) or None."""
                Wl = W_p[l]
                n_sizes = [512, 512, 512, 2] if out_dim == H else [512]
                psum = [
                    ps.tile([BS, n], F32, tag=f"ps{n_i}", name=f"ps{l}_{n_i}")
                    for n_i, n in enumerate(n_sizes)
                ]
                wt2 = None
                if in_tail is not None:
                    # K-tail rows 1536:1539 of W (w1536, w1537, bias) in one
                    # DMA, issued BEFORE the big tiles so the group-closing
                    # tail matmul's data is never stuck behind them in the
                    # ring FIFO.
                    wt2 = wp.tile([3, out_dim], BF16, tag="wk2", bufs=2)
                    nc.scalar.dma_start(
                        out=wt2[:], in_=Wl[in_tail[1] : in_tail[1] + 3, :]
                    )
                for xt, roff in in_tiles:
                    wt = wp.tile([128, out_dim], BF16, tag="wk")
                    _dma(out=wt[:], in_=Wl[roff : roff + 128, :])
                    off = 0
                    for n_i, n in enumerate(n_sizes):
                        nc.tensor.matmul(
                            psum[n_i][:],
                            xt[:],
                            wt[:, off : off + n],
                            start=(roff == in_tiles[0][1]),
                            stop=False,
                        )
                        off += n
                if in_tail is not None:
                    xt2, roff2 = in_tail
                    off = 0
                    for n_i, n in enumerate(n_sizes):
                        nc.tensor.matmul(
                            psum[n_i][:], xt2[:], wt2[:, off : off + n],
                            start=False, stop=True,
                        )
                        off += n
                else:
                    # L0: bias via ones-row (K=1) matmul
                    off = 0
                    for n_i, n in enumerate(n_sizes):
                        nc.tensor.matmul(
                            psum[n_i][:], ones[:], bias[:, off : off + n],
                            start=False, stop=True,
                        )
                        off += n
                h = io.tile([BS, out_dim + 1], out_dt, tag=f"h{l % 2}")
                off = 0
                for n_i, n in enumerate(n_sizes):
                    nc.scalar.activation(h[:, off : off + n], psum[n_i][:], act_fn)
                    off += n
                return h

            def transpose_h(h, l):
                # h is [64, H+1]; column H is memset to 1.0 so the K-tail
                # transpose [64, 1536:1539] yields (h1536, h1537, ones) rows.
                nc.vector.memset(h[:, H : H + 1], 1.0)
                tiles = []
                for j in range(12):
                    p = pst.tile([128, 64], BF16, tag="pstT")
                    nc.tensor.transpose(p[:], h[:, j * 128 : (j + 1) * 128], id64b[:])
                    ht = htp.tile([128, 64], BF16, tag=f"hT{j}")
                    nc.vector.tensor_copy(ht[:], p[:])
                    tiles.append((ht, j * 128))
                p2 = pst.tile([3, 64], BF16, tag="pstA", bufs=1)
                nc.tensor.transpose(p2[:], h[:, 1536 : H + 1], id64b[:])
                ht2 = htp.tile([3, 64], BF16, tag="hTtail")
                nc.vector.tensor_copy(ht2[:], p2[:])
                return tiles, (ht2, 1536)

            l0_tiles = (
                [(xC[j], j * 128) for j in range(4)]
                + [(xS[j], 512 + j * 128) for j in range(4)]
                + [(xF[j], 1025 + j * 128) for j in range(4)]
            )
            h = mlp_layer(0, l0_tiles, None, H, b0p, AF.Tanh, BF16)
            # ---- A transposed: AT_j [128, 512] bf16, partition = col of A ----
            AT = [
                atp.tile([128, DIM], BF16, tag=f"AT{j}", name=f"AT{j}")
                for j in range(4)
            ]
            for i in range(4):
                arow = ain.tile([128, DIM], BF16, tag="arow")
                _dma(out=arow[:], in_=A_p[i * 128 : (i + 1) * 128, :])
                for j in range(4):
                    p = pst.tile([128, 128], BF16, tag="pstA", bufs=1)
                    nc.tensor.transpose(
                        p[:], arow[:, j * 128 : (j + 1) * 128], id128b[:]
                    )
                    nc.vector.tensor_copy(AT[j][:, i * 128 : (i + 1) * 128], p[:])

            # ---- forcesum = C*(S@A^T) - S*(C@A^T) ----
            # AS[b,i] = sum_j S[b,j] A[i,j]:  lhsT=xS_j [128,64], rhs=AT_j [128,512]
            fs = io.tile([BS, DIM], F32, tag="fs")
            for name, xt in (("AS", xS), ("AC", xC)):
                ptr = pst.tile([BS, DIM], F32, tag="pstrig", bufs=1)
                for j in range(4):
                    nc.tensor.matmul(
                        ptr[:], xt[j][:], AT[j][:], start=(j == 0), stop=(j == 3)
                    )
                if name == "AS":
                    nc.vector.tensor_mul(fs[:], C[:], ptr[:])
                else:
                    tmp = io.tile([BS, DIM], F32, tag="fs2")
                    nc.vector.tensor_mul(tmp[:], S[:], ptr[:])
                    nc.vector.tensor_sub(fs[:], fs[:], tmp[:])

            for l in (1, 2):
                tiles, tail = transpose_h(h, l)
                h = mlp_layer(l, tiles, tail, H, None, AF.Tanh, BF16)
            tiles, tail = transpose_h(h, 3)
            cforce = mlp_layer(3, tiles, tail, DIM, None, AF.Copy, F32)

            # ---- outputs ----
            out_sb = io.tile([BS, DIM + 1], F32, tag="osb")
            # force = cforce * fs / DIM + freqs
            fm = io.tile([BS, DIM], F32, tag="fm")
            nc.vector.tensor_mul(fm[:], cforce[:, 0:DIM], fs[:])
            nc.vector.tensor_scalar_mul(fm[:], fm[:], 1.0 / DIM)
            nc.vector.tensor_add(out_sb[:, 0:DIM], fm[:], fr[:])
            # f1 = sum_i cforce^2
            sq = io.tile([BS, DIM], F32, tag="sq")
            nc.scalar.activation(
                sq[:], cforce[:, 0:DIM], AF.Square, accum_out=out_sb[:, DIM : DIM + 1]
            )
            nc.sync.dma_start(out=out_p[:], in_=out_sb[:])

        if loop_reps is not None:
            with tc.For_i(0, loop_reps, 1):
                _emit(0)
        else:
            for _rep in range(reps):
                _emit(_rep)

    _split_waits(nc)
    return nc


def make_in_maps(inputs):
    bf = ml_dtypes.bfloat16
    shared = {"t": np.ascontiguousarray(inputs["t"], dtype=np.float32)}
    for k in ("A", "W0", "b0"):
        shared[k] = np.ascontiguousarray(np.asarray(inputs[k]).astype(bf))
    for k in ("W1", "W2", "W3"):
        W = np.asarray(inputs[k], dtype=np.float32)
        b = np.asarray(inputs["b" + k[1]], dtype=np.float32)
        shared[k] = np.ascontiguousarray(np.vstack([W, b[None, :]]).astype(bf))
    y = np.asarray(inputs["y"], dtype=np.float32)
    freqs = np.asarray(inputs["freqs"], dtype=np.float32)
    in_maps = []
    for i in range(NCORES):
        m = dict(shared)
        m["y"] = np.ascontiguousarray(y[i * BS : (i + 1) * BS])
        m["freqs"] = np.ascontiguousarray(freqs[i * BS : (i + 1) * BS])
        in_maps.append(m)
    return in_maps


_NC_CACHE = {}


def kernel(**inputs):
    key = "nc"
    if key not in _NC_CACHE:
        _NC_CACHE[key] = _build()
    nc = _NC_CACHE[key]

    in_maps = make_in_maps(inputs)
    res = run_bass_kernel_spmd(nc, in_maps, core_ids=list(range(NCORES)))
    out = np.concatenate([res.results[i]["out"] for i in range(NCORES)], axis=0)
    return out.astype(np.float32)


# revision 28
# speedup vs baseline: 1.1295x; 1.1295x over previous
"""Trainium2 Bass kernel for nn_KuramotoHyperUniversal.

Data-parallel over batch across 8 NeuronCores (64 rows/core); weights
replicated. The (B,D,D) pairwise term is computed via the identity
  sum_j sin(y_j - y_i) A[i,j] = cos(y_i)*(A@sin(y))_i - sin(y_i)*(A@cos(y))_i
so it becomes two [64,512]x[512,512] matmuls instead of a 64MB tensor.
The constant t-column of the MLP input is folded into the layer-0 bias.

Memory-bound on weight streaming; the key optimizations vs a naive port:
- Weights and A are host-cast to bf16 (halves HBM bytes; matmuls run
  bf16 x bf16 with fp32 PSUM accumulation; rel err ~7.4e-3 vs 2e-2 gate).
- One full-width DMA per 128-row weight K-tile (394KB), alternated
  between the two HWDGE rings (SP via nc.sync, ACT via nc.scalar) —
  each dma_start costs ~0.6us of ring occupancy, so DMA count and
  single-ring serialization dominate before bandwidth does.
- Biases of layers 1-3 are appended as an extra row of W on the host,
  so the K-tail (rows 1536:1538 + bias) arrives in ONE small DMA that
  is issued BEFORE the layer's big tiles (ring FIFO: the group-closing
  tail matmul must not wait behind prefetched next-layer DMAs), and the
  bias-add folds into the tail matmul (the rhs tail tile carries a ones
  row produced by a ones-column appended to h before transposing).
- w_bufs=12 weight prefetch depth measured optimal (deeper prefetch
  delays current-layer tiles in the ring FIFO and is strictly worse).

Measured ~49-52us per invocation on-device (loop-slope method), vs
139.6us for the fp32r two-DMAs-per-tile single-ring baseline.
"""

import numpy as np
import ml_dtypes
from contextlib import ExitStack

import concourse.bass as bass
import concourse.mybir as mybir
import concourse.tile as tile
from concourse.vector_clock import ScopedClock, VectorClock
from concourse.bass_utils import run_bass_kernel_spmd
from concourse.masks import make_identity

DIM = 512
BATCH = 512
NCORES = 8
BS = BATCH // NCORES  # 64
H = 2 + 3 * DIM  # 1538
IN_SZ = 1 + 3 * DIM  # 1537
F32 = mybir.dt.float32
BF16 = mybir.dt.bfloat16
PI_HALF = float(np.pi / 2.0)


def _split_drain_and_barrier(self, tick_clock, wait_clock):
    # Walrus in this container rejects >2 sync waits on one CTRL (drain)
    # instruction; emit one single-wait NOP per outstanding proc instead.
    gc = tick_clock.global_clock
    ticks = list(gc)
    for p, t in enumerate(ticks):
        if t > 0:
            v = [0] * len(ticks)
            v[p] = t
            nop = self.nc.sync.nop(nofuse=True, hint=f"drain_wait_{p}")
            wait_clock.add_sem_waits(nop.ins, ScopedClock({None: VectorClock(v)}))
    self.nc.sync.drain()
    self.nc.all_engine_barrier()
    popped = self.nc._tile_sem_poison_stack.pop()
    assert popped is self._sem_poison
    self.nc.clear_and_free_semaphores(list(self.sems.allocated().values()))
    self.nc.all_engine_barrier()


tile.TileContext._drain_and_barrier = _split_drain_and_barrier


_MAX_WAITS = 1


def _split_waits(nc, limit=_MAX_WAITS):
    """Walrus rejects instructions carrying more than `limit` sync waits;
    move the excess onto same-engine NOPs inserted just before."""
    import bass_rust

    n = 0
    for f in nc.m.functions:
        for bb in f.blocks:
            out = []
            for inst in bb.instructions:
                si = inst.sync_info
                if si is not None and si.on_wait and len(si.on_wait) > limit:
                    waits = list(si.on_wait)
                    extra, keep = waits[:-limit], waits[-limit:]
                    for i in range(0, len(extra), limit):
                        nop = mybir.InstNoOp(name=f"I-wsplit-{n}", engine=inst.engine)
                        n += 1
                        nop.sync_info = bass_rust.SyncInfo(
                            on_wait=extra[i : i + limit], on_update=[]
                        )
                        out.append(nop)
                    inst.sync_info = bass_rust.SyncInfo(
                        on_wait=keep, on_update=list(si.on_update)
                    )
                out.append(inst)
            bb.instructions = out


def _build(w_bufs=12, reps=1, loop_reps=None):
    nc = bass.Bass()
    AF = mybir.ActivationFunctionType

    t_p = nc.declare_dram_parameter("t", [1], F32, isOutput=False)
    y_p = nc.declare_dram_parameter("y", [BS, DIM + 1], F32, isOutput=False)
    fr_p = nc.declare_dram_parameter("freqs", [BS, DIM], F32, isOutput=False)
    A_p = nc.declare_dram_parameter("A", [DIM, DIM], BF16, isOutput=False)
    # W1..W3 carry their bias appended as a final row (host-side vstack),
    # so the K-tail rows 1536:1539 (w1536, w1537, bias) arrive in one DMA
    # and the bias-add folds into the tail matmul (rhs tile has a ones row).
    W_p = [
        nc.declare_dram_parameter("W0", [IN_SZ, H], BF16, isOutput=False),
        nc.declare_dram_parameter("W1", [H + 1, H], BF16, isOutput=False),
        nc.declare_dram_parameter("W2", [H + 1, H], BF16, isOutput=False),
        nc.declare_dram_parameter("W3", [H + 1, DIM], BF16, isOutput=False),
    ]
    b0_p = nc.declare_dram_parameter("b0", [H], BF16, isOutput=False)
    out_p = nc.declare_dram_parameter("out", [BS, DIM + 1], F32, isOutput=True)

    with ExitStack() as ctx:
        tc = ctx.enter_context(tile.TileContext(nc))
        const = ctx.enter_context(tc.tile_pool(name="const", bufs=1))
        io = ctx.enter_context(tc.tile_pool(name="io", bufs=1))
        xtp = ctx.enter_context(tc.tile_pool(name="xtp", bufs=1))
        atp = ctx.enter_context(tc.tile_pool(name="atp", bufs=1))
        htp = ctx.enter_context(tc.tile_pool(name="htp", bufs=2))
        wp = ctx.enter_context(tc.tile_pool(name="wp", bufs=w_bufs))
        ain = ctx.enter_context(tc.tile_pool(name="ain", bufs=2))
        ps = ctx.enter_context(tc.tile_pool(name="ps", bufs=1, space="PSUM"))
        pst = ctx.enter_context(tc.tile_pool(name="pst", bufs=2, space="PSUM"))

        id64 = const.tile([64, 64], F32, tag="id64")
        make_identity(nc, id64[:])
        id64b = const.tile([64, 64], BF16, tag="id64b")
        nc.vector.tensor_copy(id64b[:], id64[:])
        id128f = const.tile([128, 128], F32, tag="id128f")
        make_identity(nc, id128f[:])
        id128b = const.tile([128, 128], BF16, tag="id128b")
        nc.vector.tensor_copy(id128b[:], id128f[:])
        ones = const.tile([1, 64], BF16, tag="ones")
        nc.vector.memset(ones[:], 1.0)
        pih = const.tile([BS, 1], F32, tag="pih")
        nc.vector.memset(pih[:], PI_HALF)

        # Alternate weight DMAs between the two HWDGE rings (SP via nc.sync,
        # ACT via nc.scalar) — each dma_start occupies its ring for ~0.6us
        # fixed cost, so a single ring serializes on DMA count.
        _ring = [0]

        def _dma(out, in_):
            eng = nc.sync if _ring[0] % 2 == 0 else nc.scalar
            _ring[0] += 1
            eng.dma_start(out=out, in_=in_)

        def _emit(rep):
            # ---- inputs ----
            yd = io.tile([BS, DIM], F32, tag="yd")
            nc.scalar.dma_start(out=yd[:], in_=y_p[:, 0:DIM])
            fr = io.tile([BS, DIM], F32, tag="fr")
            nc.scalar.dma_start(out=fr[:], in_=fr_p[:])
            t_sb = const.tile([1, 1], F32, tag="t")
            nc.scalar.dma_start(out=t_sb[:], in_=t_p[None, :])

            # C = cos(yd) = sin(yd + pi/2), S = sin(yd)   [64, 512]
            C = io.tile([BS, DIM], F32, tag="C")
            nc.scalar.activation(C[:], yd[:], AF.Sin, bias=pih[:])
            S = io.tile([BS, DIM], F32, tag="S")
            nc.scalar.activation(S[:], yd[:], AF.Sin)

            # ---- transposed copies (feature-on-partition, [128, 64] bf16) ----
            def transpose4(src, pref):
                tiles = []
                for j in range(4):
                    p = pst.tile([128, 64], F32, tag="pstT")
                    nc.tensor.transpose(p[:], src[:, j * 128 : (j + 1) * 128], id64[:])
                    tt = xtp.tile([128, 64], BF16, tag=f"{pref}{j}")
                    nc.vector.tensor_copy(tt[:], p[:])
                    tiles.append(tt)
                return tiles

            xC = transpose4(C, "xC")
            xS = transpose4(S, "xS")
            xF = transpose4(fr, "xF")

            # ---- biases: b0 row for the L0 ones-matmul; b1..b3 are folded
            # into the K-tail weight tiles inside mlp_layer ----
            b0row = const.tile([1, H], BF16, tag="b0", name="b0row")
            nc.scalar.dma_start(out=b0row[:], in_=b0_p[None, :])
            # b0' = b0 + (t-1) * W0[1024, :]
            w0row = const.tile([1, H], BF16, tag="w0row")
            nc.scalar.dma_start(out=w0row[:], in_=W_p[0][1024:1025, :])
            tm1 = const.tile([1, 1], F32, tag="tm1")
            nc.vector.tensor_scalar_add(tm1[:], t_sb[:], -1.0)
            b0p = const.tile([1, H], BF16, tag="b0p")
            nc.vector.tensor_scalar_mul(b0p[:], w0row[:], tm1[:])
            nc.vector.tensor_add(b0p[:], b0p[:], b0row[:])

            # ---- MLP ----
            def mlp_layer(l, in_tiles, in_tail, out_dim, bias, act_fn, out_dt):
                """in_tiles: list of (xT_tile[128,64] bf16, W row offset);
                in_tail: ([2,64] tile, row offset) or None."""
                Wl = W_p[l]
                n_sizes = [512, 512, 512, 2] if out_dim == H else [512]
                psum = [
                    ps.tile([BS, n], F32, tag=f"ps{n_i}", name=f"ps{l}_{n_i}")
                    for n_i, n in enumerate(n_sizes)
                ]
                wt2 = None
                if in_tail is not None:
                    # K-tail rows 1536:1539 of W (w1536, w1537, bias) in one
                    # DMA, issued BEFORE the big tiles so the group-closing
                    # tail matmul's data is never stuck behind them in the
                    # ring FIFO.
                    wt2 = wp.tile([3, out_dim], BF16, tag="wk2", bufs=2)
                    nc.scalar.dma_start(
                        out=wt2[:], in_=Wl[in_tail[1] : in_tail[1] + 3, :]
                    )
                for xt, roff in in_tiles:
                    wt = wp.tile([128, out_dim], BF16, tag="wk")
                    _dma(out=wt[:], in_=Wl[roff : roff + 128, :])
                    off = 0
                    for n_i, n in enumerate(n_sizes):
                        nc.tensor.matmul(
                            psum[n_i][:],
                            xt[:],
                            wt[:, off : off + n],
                            start=(roff == in_tiles[0][1]),
                            stop=False,
                        )
                        off += n
                if in_tail is not None:
                    xt2, roff2 = in_tail
                    off = 0
                    for n_i, n in enumerate(n_sizes):
                        nc.tensor.matmul(
                            psum[n_i][:], xt2[:], wt2[:, off : off + n],
                            start=False, stop=True,
                        )
                        off += n
                else:
                    # L0: bias via ones-row (K=1) matmul
                    off = 0
                    for n_i, n in enumerate(n_sizes):
                        nc.tensor.matmul(
                            psum[n_i][:], ones[:], bias[:, off : off + n],
                            start=False, stop=True,
                        )
                        off += n
                h = io.tile([BS, out_dim + 1], out_dt, tag=f"h{l % 2}")
                off = 0
                for n_i, n in enumerate(n_sizes):
                    nc.scalar.activation(h[:, off : off + n], psum[n_i][:], act_fn)
                    off += n
                return h

            def transpose_h(h, l):
                # h is [64, H+1]; column H is memset to 1.0 so the K-tail
                # transpose [64, 1536:1539] yields (h1536, h1537, ones) rows.
                nc.vector.memset(h[:, H : H + 1], 1.0)
                tiles = []
                for j in range(12):
                    p = pst.tile([128, 64], BF16, tag="pstT")
                    nc.tensor.transpose(p[:], h[:, j * 128 : (j + 1) * 128], id64b[:])
                    ht = htp.tile([128, 64], BF16, tag=f"hT{j}")
                    nc.vector.tensor_copy(ht[:], p[:])
                    tiles.append((ht, j * 128))
                p2 = pst.tile([3, 64], BF16, tag="pstA", bufs=1)
                nc.tensor.transpose(p2[:], h[:, 1536 : H + 1], id64b[:])
                ht2 = htp.tile([3, 64], BF16, tag="hTtail")
                nc.vector.tensor_copy(ht2[:], p2[:])
                return tiles, (ht2, 1536)

            l0_tiles = (
                [(xC[j], j * 128) for j in range(4)]
                + [(xS[j], 512 + j * 128) for j in range(4)]
                + [(xF[j], 1025 + j * 128) for j in range(4)]
            )
            h = mlp_layer(0, l0_tiles, None, H, b0p, AF.Tanh, BF16)


            def trig_half(name, xt_tiles):
                ptr = pst.tile([BS, DIM], F32, tag="pstrig", bufs=1, name=f"ptr{name}")
                for j in range(4):
                    nc.tensor.matmul(
                        ptr[:], xt_tiles[j][:], AT[j][:], start=(j == 0), stop=(j == 3)
                    )
                if name == "AS":
                    nc.vector.tensor_mul(fs[:], C[:], ptr[:])
                else:
                    tmp = io.tile([BS, DIM], F32, tag="fs2")
                    nc.vector.tensor_mul(tmp[:], S[:], ptr[:])
                    nc.vector.tensor_sub(fs[:], fs[:], tmp[:])

            # ---- A transposed: AT_j [128, 512] bf16, partition = col of A ----
            AT = [
                atp.tile([128, DIM], BF16, tag=f"AT{j}", name=f"AT{j}")
                for j in range(4)
            ]
            for i in range(4):
                arow = ain.tile([128, DIM], BF16, tag="arow")
                _dma(out=arow[:], in_=A_p[i * 128 : (i + 1) * 128, :])
                for j in range(4):
                    p = pst.tile([128, 128], BF16, tag="pstA", bufs=1)
                    nc.tensor.transpose(
                        p[:], arow[:, j * 128 : (j + 1) * 128], id128b[:]
                    )
                    nc.vector.tensor_copy(AT[j][:, i * 128 : (i + 1) * 128], p[:])

            tiles, tail = transpose_h(h, 1)
            h = mlp_layer(1, tiles, tail, H, None, AF.Tanh, BF16)
            # fs = C*(S@A^T): the AS matmuls fill the L1->L2 boundary stall
            fs = io.tile([BS, DIM], F32, tag="fs")
            trig_half("AS", xS)
            tiles, tail = transpose_h(h, 2)
            h = mlp_layer(2, tiles, tail, H, None, AF.Tanh, BF16)
            # fs -= S*(C@A^T): the AC matmuls fill the L2->L3 boundary stall
            trig_half("AC", xC)
            tiles, tail = transpose_h(h, 3)
            cforce = mlp_layer(3, tiles, tail, DIM, None, AF.Copy, F32)

            # ---- outputs ----
            out_sb = io.tile([BS, DIM + 1], F32, tag="osb")
            # force = cforce * fs / DIM + freqs
            fm = io.tile([BS, DIM], F32, tag="fm")
            nc.vector.tensor_mul(fm[:], cforce[:, 0:DIM], fs[:])
            nc.vector.tensor_scalar_mul(fm[:], fm[:], 1.0 / DIM)
            nc.vector.tensor_add(out_sb[:, 0:DIM], fm[:], fr[:])
            # f1 = sum_i cforce^2
            sq = io.tile([BS, DIM], F32, tag="sq")
            nc.scalar.activation(
                sq[:], cforce[:, 0:DIM], AF.Square, accum_out=out_sb[:, DIM : DIM + 1]
            )
            nc.sync.dma_start(out=out_p[:], in_=out_sb[:])

        if loop_reps is not None:
            with tc.For_i(0, loop_reps, 1):
                _emit(0)
        else:
            for _rep in range(reps):
                _emit(_rep)

    _split_waits(nc)
    return nc


def make_in_maps(inputs):
    bf = ml_dtypes.bfloat16
    shared = {"t": np.ascontiguousarray(inputs["t"], dtype=np.float32)}
    for k in ("A", "W0", "b0"):
        shared[k] = np.ascontiguousarray(np.asarray(inputs[k]).astype(bf))
    for k in ("W1", "W2", "W3"):
        W = np.asarray(inputs[k], dtype=np.float32)
        b = np.asarray(inputs["b" + k[1]], dtype=np.float32)
        shared[k] = np.ascontiguousarray(np.vstack([W, b[None, :]]).astype(bf))
    y = np.asarray(inputs["y"], dtype=np.float32)
    freqs = np.asarray(inputs["freqs"], dtype=np.float32)
    in_maps = []
    for i in range(NCORES):
        m = dict(shared)
        m["y"] = np.ascontiguousarray(y[i * BS : (i + 1) * BS])
        m["freqs"] = np.ascontiguousarray(freqs[i * BS : (i + 1) * BS])
        in_maps.append(m)
    return in_maps


_NC_CACHE = {}


def kernel(**inputs):
    key = "nc"
    if key not in _NC_CACHE:
        _NC_CACHE[key] = _build()
    nc = _NC_CACHE[key]

    in_maps = make_in_maps(inputs)
    res = run_bass_kernel_spmd(nc, in_maps, core_ids=list(range(NCORES)))
    out = np.concatenate([res.results[i]["out"] for i in range(NCORES)], axis=0)
    return out.astype(np.float32)


# revision 38
# speedup vs baseline: 1.2783x; 1.1317x over previous
"""Trainium2 Bass kernel for nn_KuramotoHyperUniversal.

Data-parallel over batch across 8 NeuronCores (64 rows/core); weights
replicated. The (B,D,D) pairwise term is computed via the identity
  sum_j sin(y_j - y_i) A[i,j] = cos(y_i)*(A@sin(y))_i - sin(y_i)*(A@cos(y))_i
so it becomes two [64,512]x[512,512] matmuls instead of a 64MB tensor.
The constant t-column of the MLP input is folded into the layer-0 bias.

Memory-bound on weight streaming; the key optimizations vs a naive port:
- Weights and A are host-cast to bf16 (halves HBM bytes; matmuls run
  bf16 x bf16 with fp32 PSUM accumulation; rel err ~7.4e-3 vs 2e-2 gate).
- One 788KB DMA per PAIR of consecutive 128-row weight K-tiles (3-dim
  access pattern: partition p carries W rows roff+p and roff+128+p),
  alternated 1:1 between the two HWDGE rings (SP via nc.sync, ACT via
  nc.scalar) — each dma_start costs fixed ring-issue occupancy, so DMA
  count and single-ring serialization dominate before bandwidth does
  (halving the count 51->27 measured ~15us faster than singles in a
  slow-terminal session; ring ratio must stay strictly 1:1).
- Biases of layers 1-3 are appended as an extra row of W on the host,
  so the K-tail (rows 1536:1538 + bias) arrives in ONE small DMA that
  is issued BEFORE the layer's big tiles (ring FIFO: the group-closing
  tail matmul must not wait behind prefetched next-layer DMAs), and the
  bias-add folds into the tail matmul (the rhs tail tile carries a ones
  row produced by a ones-column appended to h before transposing).
- w_bufs=6 pair-tiles (= 12 K-tiles) prefetch depth; deeper prefetch
  delays current-layer tiles in the ring FIFO and is strictly worse.
- The kernel is latency-bound above the ~31us pure-DMA floor: each layer
  boundary stalls the in-order PE stream on the ACT->transpose->lhsT
  chain. The independent trig-part work (A transposes, S@A^T, C@A^T)
  is distributed as boundary FILLER: A transposes between L0->L1, the
  AS matmuls between L1->L2, the AC matmuls between L2->L3 (moving all
  of it after L3 measures 60us; bunching it at one boundary, 49us;
  distributed, 47-48us).

Measured ~47-48us per invocation on-device (loop-slope method), vs
139.6us for the fp32r two-DMAs-per-tile single-ring baseline.
"""

import numpy as np
import ml_dtypes
from contextlib import ExitStack

import concourse.bass as bass
import concourse.mybir as mybir
import concourse.tile as tile
from concourse.vector_clock import ScopedClock, VectorClock
from concourse.bass_utils import run_bass_kernel_spmd
from concourse.masks import make_identity

DIM = 512
BATCH = 512
NCORES = 8
BS = BATCH // NCORES  # 64
H = 2 + 3 * DIM  # 1538
IN_SZ = 1 + 3 * DIM  # 1537
F32 = mybir.dt.float32
BF16 = mybir.dt.bfloat16
PI_HALF = float(np.pi / 2.0)


def _split_drain_and_barrier(self, tick_clock, wait_clock):
    # Walrus in this container rejects >2 sync waits on one CTRL (drain)
    # instruction; emit one single-wait NOP per outstanding proc instead.
    gc = tick_clock.global_clock
    ticks = list(gc)
    for p, t in enumerate(ticks):
        if t > 0:
            v = [0] * len(ticks)
            v[p] = t
            nop = self.nc.sync.nop(nofuse=True, hint=f"drain_wait_{p}")
            wait_clock.add_sem_waits(nop.ins, ScopedClock({None: VectorClock(v)}))
    self.nc.sync.drain()
    self.nc.all_engine_barrier()
    popped = self.nc._tile_sem_poison_stack.pop()
    assert popped is self._sem_poison
    self.nc.clear_and_free_semaphores(list(self.sems.allocated().values()))
    self.nc.all_engine_barrier()


tile.TileContext._drain_and_barrier = _split_drain_and_barrier


_MAX_WAITS = 1


def _split_waits(nc, limit=_MAX_WAITS):
    """Walrus rejects instructions carrying more than `limit` sync waits;
    move the excess onto same-engine NOPs inserted just before."""
    import bass_rust

    n = 0
    for f in nc.m.functions:
        for bb in f.blocks:
            out = []
            for inst in bb.instructions:
                si = inst.sync_info
                if si is not None and si.on_wait and len(si.on_wait) > limit:
                    waits = list(si.on_wait)
                    extra, keep = waits[:-limit], waits[-limit:]
                    for i in range(0, len(extra), limit):
                        nop = mybir.InstNoOp(name=f"I-wsplit-{n}", engine=inst.engine)
                        n += 1
                        nop.sync_info = bass_rust.SyncInfo(
                            on_wait=extra[i : i + limit], on_update=[]
                        )
                        out.append(nop)
                    inst.sync_info = bass_rust.SyncInfo(
                        on_wait=keep, on_update=list(si.on_update)
                    )
                out.append(inst)
            bb.instructions = out


def _build(w_bufs=6, reps=1, loop_reps=None):
    nc = bass.Bass()
    AF = mybir.ActivationFunctionType

    t_p = nc.declare_dram_parameter("t", [1], F32, isOutput=False)
    y_p = nc.declare_dram_parameter("y", [BS, DIM + 1], F32, isOutput=False)
    fr_p = nc.declare_dram_parameter("freqs", [BS, DIM], F32, isOutput=False)
    A_p = nc.declare_dram_parameter("A", [DIM, DIM], BF16, isOutput=False)
    # W1..W3 carry their bias appended as a final row (host-side vstack),
    # so the K-tail rows 1536:1539 (w1536, w1537, bias) arrive in one DMA
    # and the bias-add folds into the tail matmul (rhs tile has a ones row).
    W_p = [
        nc.declare_dram_parameter("W0", [IN_SZ, H], BF16, isOutput=False),
        nc.declare_dram_parameter("W1", [H + 1, H], BF16, isOutput=False),
        nc.declare_dram_parameter("W2", [H + 1, H], BF16, isOutput=False),
        nc.declare_dram_parameter("W3", [H + 1, DIM], BF16, isOutput=False),
    ]
    b0_p = nc.declare_dram_parameter("b0", [H], BF16, isOutput=False)
    out_p = nc.declare_dram_parameter("out", [BS, DIM + 1], F32, isOutput=True)

    with ExitStack() as ctx:
        tc = ctx.enter_context(tile.TileContext(nc))
        const = ctx.enter_context(tc.tile_pool(name="const", bufs=1))
        io = ctx.enter_context(tc.tile_pool(name="io", bufs=1))
        xtp = ctx.enter_context(tc.tile_pool(name="xtp", bufs=1))
        atp = ctx.enter_context(tc.tile_pool(name="atp", bufs=1))
        htp = ctx.enter_context(tc.tile_pool(name="htp", bufs=2))
        wp = ctx.enter_context(tc.tile_pool(name="wp", bufs=w_bufs))
        ain = ctx.enter_context(tc.tile_pool(name="ain", bufs=2))
        ps = ctx.enter_context(tc.tile_pool(name="ps", bufs=1, space="PSUM"))
        pst = ctx.enter_context(tc.tile_pool(name="pst", bufs=2, space="PSUM"))

        id64 = const.tile([64, 64], F32, tag="id64")
        make_identity(nc, id64[:])
        id64b = const.tile([64, 64], BF16, tag="id64b")
        nc.vector.tensor_copy(id64b[:], id64[:])
        id128f = const.tile([128, 128], F32, tag="id128f")
        make_identity(nc, id128f[:])
        id128b = const.tile([128, 128], BF16, tag="id128b")
        nc.vector.tensor_copy(id128b[:], id128f[:])
        ones = const.tile([1, 64], BF16, tag="ones")
        nc.vector.memset(ones[:], 1.0)
        pih = const.tile([BS, 1], F32, tag="pih")
        nc.vector.memset(pih[:], PI_HALF)

        # Alternate weight DMAs between the two HWDGE rings (SP via nc.sync,
        # ACT via nc.scalar) — each dma_start occupies its ring for ~0.6us
        # fixed cost, so a single ring serializes on DMA count.
        _ring = [0]

        def _dma(out, in_):
            eng = nc.sync if _ring[0] % 2 == 0 else nc.scalar
            _ring[0] += 1
            eng.dma_start(out=out, in_=in_)

        def _emit(rep):
            # ---- inputs ----
            yd = io.tile([BS, DIM], F32, tag="yd")
            nc.scalar.dma_start(out=yd[:], in_=y_p[:, 0:DIM])
            fr = io.tile([BS, DIM], F32, tag="fr")
            nc.scalar.dma_start(out=fr[:], in_=fr_p[:])
            t_sb = const.tile([1, 1], F32, tag="t")
            nc.scalar.dma_start(out=t_sb[:], in_=t_p[None, :])

            # C = cos(yd) = sin(yd + pi/2), S = sin(yd)   [64, 512]
            C = io.tile([BS, DIM], F32, tag="C")
            nc.scalar.activation(C[:], yd[:], AF.Sin, bias=pih[:])
            S = io.tile([BS, DIM], F32, tag="S")
            nc.scalar.activation(S[:], yd[:], AF.Sin)

            # ---- transposed copies (feature-on-partition, [128, 64] bf16) ----
            def transpose4(src, pref):
                tiles = []
                for j in range(4):
                    p = pst.tile([128, 64], F32, tag="pstT")
                    nc.tensor.transpose(p[:], src[:, j * 128 : (j + 1) * 128], id64[:])
                    tt = xtp.tile([128, 64], BF16, tag=f"{pref}{j}")
                    nc.vector.tensor_copy(tt[:], p[:])
                    tiles.append(tt)
                return tiles

            xC = transpose4(C, "xC")
            xS = transpose4(S, "xS")
            xF = transpose4(fr, "xF")

            # ---- biases: b0 row for the L0 ones-matmul; b1..b3 are folded
            # into the K-tail weight tiles inside mlp_layer ----
            b0row = const.tile([1, H], BF16, tag="b0", name="b0row")
            nc.scalar.dma_start(out=b0row[:], in_=b0_p[None, :])
            # b0' = b0 + (t-1) * W0[1024, :]
            w0row = const.tile([1, H], BF16, tag="w0row")
            nc.scalar.dma_start(out=w0row[:], in_=W_p[0][1024:1025, :])
            tm1 = const.tile([1, 1], F32, tag="tm1")
            nc.vector.tensor_scalar_add(tm1[:], t_sb[:], -1.0)
            b0p = const.tile([1, H], BF16, tag="b0p")
            nc.vector.tensor_scalar_mul(b0p[:], w0row[:], tm1[:])
            nc.vector.tensor_add(b0p[:], b0p[:], b0row[:])

            # ---- MLP ----
            def mlp_layer(l, in_tiles, in_tail, out_dim, bias, act_fn, out_dt):
                """in_tiles: list of (xT_tile[128,64] bf16, W row offset);
                in_tail: ([2,64] tile, row offset) or None."""
                Wl = W_p[l]
                n_sizes = [512, 512, 512, 2] if out_dim == H else [512]
                psum = [
                    ps.tile([BS, n], F32, tag=f"ps{n_i}", name=f"ps{l}_{n_i}")
                    for n_i, n in enumerate(n_sizes)
                ]
                wt2 = None
                if in_tail is not None:
                    # K-tail rows 1536:1539 of W (w1536, w1537, bias) in one
                    # DMA, issued BEFORE the big tiles so the group-closing
                    # tail matmul's data is never stuck behind them in the
                    # ring FIFO.
                    wt2 = wp.tile([3, out_dim], BF16, tag="wk2", bufs=2)
                    nc.scalar.dma_start(
                        out=wt2[:], in_=Wl[in_tail[1] : in_tail[1] + 3, :]
                    )
                # consecutive K-tile PAIRS share one 788KB DMA (3-dim AP:
                # partition p carries W rows roff+p and roff+128+p) — halves
                # the per-DMA ring-issue overhead at unchanged wire bytes.
                for pi in range(0, len(in_tiles), 2):
                    (xtA, roffA), (xtB, roffB) = in_tiles[pi], in_tiles[pi + 1]
                    assert roffB == roffA + 128
                    wt = wp.tile([128, 2, out_dim], BF16, tag="wk")
                    _dma(
                        out=wt[:],
                        in_=Wl[roffA : roffA + 256, :].rearrange(
                            "(j p) n -> p j n", p=128
                        ),
                    )
                    for j, xt in ((0, xtA), (1, xtB)):
                        off = 0
                        for n_i, n in enumerate(n_sizes):
                            nc.tensor.matmul(
                                psum[n_i][:],
                                xt[:],
                                wt[:, j : j + 1, off : off + n],
                                start=(pi == 0 and j == 0),
                                stop=False,
                            )
                            off += n
                if in_tail is not None:
                    xt2, roff2 = in_tail
                    off = 0
                    for n_i, n in enumerate(n_sizes):
                        nc.tensor.matmul(
                            psum[n_i][:], xt2[:], wt2[:, off : off + n],
                            start=False, stop=True,
                        )
                        off += n
                else:
                    # L0: bias via ones-row (K=1) matmul
                    off = 0
                    for n_i, n in enumerate(n_sizes):
                        nc.tensor.matmul(
                            psum[n_i][:], ones[:], bias[:, off : off + n],
                            start=False, stop=True,
                        )
                        off += n
                h = io.tile([BS, out_dim + 1], out_dt, tag=f"h{l % 2}")
                off = 0
                for n_i, n in enumerate(n_sizes):
                    nc.scalar.activation(h[:, off : off + n], psum[n_i][:], act_fn)
                    off += n
                return h

            def transpose_h(h, l):
                # h is [64, H+1]; column H is memset to 1.0 so the K-tail
                # transpose [64, 1536:1539] yields (h1536, h1537, ones) rows.
                nc.vector.memset(h[:, H : H + 1], 1.0)
                tiles = []
                for j in range(12):
                    p = pst.tile([128, 64], BF16, tag="pstT")
                    nc.tensor.transpose(p[:], h[:, j * 128 : (j + 1) * 128], id64b[:])
                    ht = htp.tile([128, 64], BF16, tag=f"hT{j}")
                    nc.vector.tensor_copy(ht[:], p[:])
                    tiles.append((ht, j * 128))
                p2 = pst.tile([3, 64], BF16, tag="pstA", bufs=1)
                nc.tensor.transpose(p2[:], h[:, 1536 : H + 1], id64b[:])
                ht2 = htp.tile([3, 64], BF16, tag="hTtail")
                nc.vector.tensor_copy(ht2[:], p2[:])
                return tiles, (ht2, 1536)

            l0_tiles = (
                [(xC[j], j * 128) for j in range(4)]
                + [(xS[j], 512 + j * 128) for j in range(4)]
                + [(xF[j], 1025 + j * 128) for j in range(4)]
            )
            h = mlp_layer(0, l0_tiles, None, H, b0p, AF.Tanh, BF16)


            def trig_half(name, xt_tiles):
                ptr = pst.tile([BS, DIM], F32, tag="pstrig", bufs=1, name=f"ptr{name}")
                for j in range(4):
                    nc.tensor.matmul(
                        ptr[:], xt_tiles[j][:], AT[j][:], start=(j == 0), stop=(j == 3)
                    )
                if name == "AS":
                    nc.vector.tensor_mul(fs[:], C[:], ptr[:])
                else:
                    tmp = io.tile([BS, DIM], F32, tag="fs2")
                    nc.vector.tensor_mul(tmp[:], S[:], ptr[:])
                    nc.vector.tensor_sub(fs[:], fs[:], tmp[:])

            # ---- A transposed: AT_j [128, 512] bf16, partition = col of A ----
            AT = [
                atp.tile([128, DIM], BF16, tag=f"AT{j}", name=f"AT{j}")
                for j in range(4)
            ]
            for i in range(4):
                arow = ain.tile([128, DIM], BF16, tag="arow")
                _dma(out=arow[:], in_=A_p[i * 128 : (i + 1) * 128, :])
                for j in range(4):
                    p = pst.tile([128, 128], BF16, tag="pstA", bufs=1)
                    nc.tensor.transpose(
                        p[:], arow[:, j * 128 : (j + 1) * 128], id128b[:]
                    )
                    nc.vector.tensor_copy(AT[j][:, i * 128 : (i + 1) * 128], p[:])

            tiles, tail = transpose_h(h, 1)
            h = mlp_layer(1, tiles, tail, H, None, AF.Tanh, BF16)
            # fs = C*(S@A^T): the AS matmuls fill the L1->L2 boundary stall
            fs = io.tile([BS, DIM], F32, tag="fs")
            trig_half("AS", xS)
            tiles, tail = transpose_h(h, 2)
            h = mlp_layer(2, tiles, tail, H, None, AF.Tanh, BF16)
            # fs -= S*(C@A^T): the AC matmuls fill the L2->L3 boundary stall
            trig_half("AC", xC)
            tiles, tail = transpose_h(h, 3)
            cforce = mlp_layer(3, tiles, tail, DIM, None, AF.Copy, F32)

            # ---- outputs ----
            out_sb = io.tile([BS, DIM + 1], F32, tag="osb")
            # force = cforce * fs / DIM + freqs
            fm = io.tile([BS, DIM], F32, tag="fm")
            nc.vector.tensor_mul(fm[:], cforce[:, 0:DIM], fs[:])
            nc.vector.tensor_scalar_mul(fm[:], fm[:], 1.0 / DIM)
            nc.vector.tensor_add(out_sb[:, 0:DIM], fm[:], fr[:])
            # f1 = sum_i cforce^2
            sq = io.tile([BS, DIM], F32, tag="sq")
            nc.scalar.activation(
                sq[:], cforce[:, 0:DIM], AF.Square, accum_out=out_sb[:, DIM : DIM + 1]
            )
            nc.sync.dma_start(out=out_p[:], in_=out_sb[:])

        if loop_reps is not None:
            with tc.For_i(0, loop_reps, 1):
                _emit(0)
        else:
            for _rep in range(reps):
                _emit(_rep)

    _split_waits(nc)
    return nc


def make_in_maps(inputs):
    bf = ml_dtypes.bfloat16
    shared = {"t": np.ascontiguousarray(inputs["t"], dtype=np.float32)}
    for k in ("A", "W0", "b0"):
        shared[k] = np.ascontiguousarray(np.asarray(inputs[k]).astype(bf))
    for k in ("W1", "W2", "W3"):
        W = np.asarray(inputs[k], dtype=np.float32)
        b = np.asarray(inputs["b" + k[1]], dtype=np.float32)
        shared[k] = np.ascontiguousarray(np.vstack([W, b[None, :]]).astype(bf))
    y = np.asarray(inputs["y"], dtype=np.float32)
    freqs = np.asarray(inputs["freqs"], dtype=np.float32)
    in_maps = []
    for i in range(NCORES):
        m = dict(shared)
        m["y"] = np.ascontiguousarray(y[i * BS : (i + 1) * BS])
        m["freqs"] = np.ascontiguousarray(freqs[i * BS : (i + 1) * BS])
        in_maps.append(m)
    return in_maps


_NC_CACHE = {}


def kernel(**inputs):
    key = "nc"
    if key not in _NC_CACHE:
        _NC_CACHE[key] = _build()
    nc = _NC_CACHE[key]

    in_maps = make_in_maps(inputs)
    res = run_bass_kernel_spmd(nc, in_maps, core_ids=list(range(NCORES)))
    out = np.concatenate([res.results[i]["out"] for i in range(NCORES)], axis=0)
    return out.astype(np.float32)
